# revision 4
# baseline (speedup 1.0000x reference)
"""Block-sparse linear kernel for Trainium2 — 8-core data-parallel,
PE-array tiling (8 tiles of 64x32) exploiting the 32x32 block mask.

out = 2*(x @ (weight*mask).T) + bias
x: (8, 2048, 4096) f32, weight: (4096, 4096) f32, bias: (4096,),
block_mask: (128, 128) bool over 32x32 blocks (~50% dense).

Each core handles one batch (M=2048, K=4096, N=4096). k-blocks are paired
(greedy max matching on shared-zero o-columns) into 64 K=64 "pair" units
split across the two PE row-halves; o-blocks are grouped 4-per-round into
32 j-rounds (balanced by local search). Only (pair, o-block) cells with at
least one unmasked member are computed: ~69% of the dense work. The PE runs
in 64x32 tile mode (8 concurrent tiles = row-half x col-group), raw bass
emission with ~1 semaphore per round instead of per-instruction.
PSUM: 8 banks as 4-deep round rotation x 2 row-halves. Eviction: DVE sums
the two half-banks, ACT adds bias (Identity activation, per-partition bias)
and the result is stored [o, s]-transposed; the host reassembles.
"""
import numpy as np

B, S, IN, OUT = 8, 2048, 4096, 4096
BLOCK = 32
P = 128
NJ = OUT // BLOCK          # 128 o-blocks
NK = IN // BLOCK           # 128 k-blocks
NPAIR = NK // 2            # 64 k-pairs
HALF = NPAIR // 2          # 32 pairs per PE row-half
NCG = 4                    # col-groups (o-blocks per round)
TG = NJ // NCG             # 32 j-rounds
PH = 4                     # phases
TPP = TG // PH             # 8 j-rounds per phase
NU = 4                     # s-chunks
CH = S // NU               # 512
NRND = PH * NU * TPP       # 128 (ph, u, t) rounds

LAST_EXEC_NS = None


def _plan(bm):
    """bm: [128 o-blocks, 128 k-blocks] bool -> schedule plan."""
    bm = bm.astype(np.int8)
    rng = np.random.default_rng(0)

    # --- pair k-blocks: greedy max matching on #shared-zero o-columns ---
    z = (1 - bm.T).astype(np.int32)          # [k, o]
    agree = z @ z.T
    np.fill_diagonal(agree, -1)
    order = np.dstack(np.unravel_index(np.argsort(-agree, axis=None), agree.shape))[0]
    used = np.zeros(NK, bool)
    pairs = []
    for k1, k2 in order:
        if used[k1] or used[k2]:
            continue
        used[k1] = used[k2] = True
        pairs.append((int(k1), int(k2)))
    pairs = np.array(pairs)                   # [64, 2]

    # refine matching: pair-member swaps reducing total cells
    def _ncells(P):
        return int(((bm[:, P[:, 0]] | bm[:, P[:, 1]]) > 0).sum())

    cur = _ncells(pairs)
    rng2 = np.random.default_rng(1)
    for _ in range(60000):
        p1, p2 = rng2.integers(0, NPAIR, 2)
        if p1 == p2:
            continue
        m1, m2 = rng2.integers(0, 2, 2)
        Pn = pairs.copy()
        Pn[p1, m1], Pn[p2, m2] = pairs[p2, m2], pairs[p1, m1]
        c = _ncells(Pn)
        if c <= cur:
            pairs, cur = Pn, c

    # cell[j, p] = 1 if pair p needed for o-block j
    cell = ((bm[:, pairs[:, 0]] | bm[:, pairs[:, 1]]) > 0).astype(np.int64)  # [j, pair]

    # --- assign pairs to halves + group j's into rounds (local search) ---
    half = np.zeros(NPAIR, np.int32)
    half[np.argsort(-cell.sum(0))] = np.arange(NPAIR) % 2
    nj = cell.sum(1)
    rounds = np.argsort(-nj).reshape(TG, NCG).copy()

    def lane_n(h):
        # n[j, a] = #pairs in half a needed by j
        return np.stack([cell[:, h == a].sum(1) for a in range(2)], 1)

    n = lane_n(half)
    cost = n[rounds].max(axis=(1, 2)).sum()
    for it in range(40000):
        if it % 2 == 0:
            p1, p2 = rng.integers(0, NPAIR, 2)
            a1, a2 = half[p1], half[p2]
            if a1 == a2:
                continue
            dn = np.zeros((NJ, 2), np.int64)
            dn[:, a1] = cell[:, p2] - cell[:, p1]
            dn[:, a2] = cell[:, p1] - cell[:, p2]
            n2 = n + dn
            c2 = n2[rounds].max(axis=(1, 2)).sum()
            if c2 <= cost:
                half[p1], half[p2] = a2, a1
                n, cost = n2, c2
        else:
            t1, t2 = rng.integers(0, TG, 2)
            if t1 == t2:
                continue
            c1i, c2i = rng.integers(0, NCG, 2)
            r2 = rounds.copy()
            r2[t1, c1i], r2[t2, c2i] = rounds[t2, c2i], rounds[t1, c1i]
            c2 = n[r2].max(axis=(1, 2)).sum()
            if c2 <= cost:
                rounds, cost = r2, c2

    # slots: pair -> (half a, slot index within half)
    slot_of = np.zeros(NPAIR, np.int32)
    for a in range(2):
        idx = np.where(half == a)[0]
        slot_of[idx] = np.arange(len(idx))

    # per (tg, a, c): list of pair indices (cells); pad empty lanes with a
    # dummy (first pair of the half, weights zeroed at pack time)
    cells = {}
    dummy = {}
    for tg in range(TG):
        for a in range(2):
            hp = np.where(half == a)[0]
            for c in range(NCG):
                j = rounds[tg, c]
                lst = [int(p) for p in hp if cell[j, p]]
                dum = False
                if not lst:
                    lst = [int(hp[0])]
                    dum = True
                cells[(tg, a, c)] = lst
                dummy[(tg, a, c)] = dum
    return dict(pairs=pairs, half=half, rounds=rounds, slot_of=slot_of,
                cells=cells, dummy=dummy)


def _build_program(plan):
    import concourse.bacc as bacc
    from concourse import mybir

    f16 = mybir.dt.float16
    f32 = mybir.dt.float32
    pairs, rounds = plan["pairs"], plan["rounds"]
    slot_of, cells = plan["slot_of"], plan["cells"]

    # per (a): sbuf w slot capacities per round-slot t (reused across phases)
    m = np.zeros((2, PH, TPP), np.int64)     # cells per (a, ph, t)
    for ph in range(PH):
        for t in range(TPP):
            tg = ph * TPP + t
            for a in range(2):
                m[a, ph, t] = sum(len(cells[(tg, a, c)]) for c in range(NCG))
    cap = m.max(axis=1)                       # [2, TPP]
    sb_off = np.zeros((2, TPP), np.int64)     # sbuf cell offset per (a, t)
    for a in range(2):
        sb_off[a] = np.concatenate([[0], np.cumsum(cap[a])[:-1]])
    WCAP = int(cap.sum(axis=1).max())
    # dram w offsets per (a, ph, t)
    d_off = np.zeros((2, PH, TPP), np.int64)
    for a in range(2):
        flat = m[a].reshape(-1)
        d_off[a] = np.concatenate([[0], np.cumsum(flat)[:-1]]).reshape(PH, TPP)
    WDTOT = int(m.sum(axis=(1, 2)).max())

    nc = bacc.Bacc("TRN2", debug=False, num_devices=B)
    x_d = nc.dram_tensor("xt", (P, HALF, S), f16, kind="ExternalInput")
    w_d = nc.dram_tensor("wt", (P, WDTOT, BLOCK), f16, kind="ExternalInput")
    b_d = nc.dram_tensor("bias", (P, TG), f32, kind="ExternalInput")
    o_d = nc.dram_tensor("out", (NRND, P, CH), f32, kind="ExternalOutput")

    x_sems = [nc.alloc_semaphore(f"xsem{u}") for u in range(NU)]
    w_sems = [nc.alloc_semaphore(f"wsem{t}") for t in range(TPP)]
    b_sem = nc.alloc_semaphore("bsem")
    warm_sem = nc.alloc_semaphore("warmsem")
    pe_sem = nc.alloc_semaphore("pesem")
    dve_sem = nc.alloc_semaphore("dvesem")
    act_sem = nc.alloc_semaphore("actsem")
    st_sems = [nc.alloc_semaphore(f"stsem{p}") for p in range(2)]

    with (
        nc.sbuf_tensor([P, HALF, S], f16) as xs,
        nc.sbuf_tensor([P, WCAP, BLOCK], f16) as ws,
        nc.sbuf_tensor([P, TG], f32) as bs,
        nc.sbuf_tensor([P, P], f16) as wj,
        nc.sbuf_tensor([P, 2, CH], f32) as tb,
        nc.sbuf_tensor([P, 2, CH], f32) as ob,
        nc.psum_tensor([P, CH], f32) as p0,
        nc.psum_tensor([P, CH], f32) as p1,
        nc.psum_tensor([P, CH], f32) as p2,
        nc.psum_tensor([P, CH], f32) as p3,
        nc.psum_tensor([P, CH], f32) as p4,
        nc.psum_tensor([P, CH], f32) as p5,
        nc.psum_tensor([P, CH], f32) as p6,
        nc.psum_tensor([P, CH], f32) as p7,
    ):
        psb = [p0, p1, p2, p3, p4, p5, p6, p7]

        # ---- prologue DMAs ----
        nc.sync.dma_start(out=bs[:], in_=b_d[:]).then_inc(b_sem, 16)
        # x chunks on the scalar queue (parallel with w on sync)
        for u in range(NU):
            h = HALF // 2
            nc.scalar.dma_start(
                out=xs[:, :h, u * CH:(u + 1) * CH],
                in_=x_d[:, :h, u * CH:(u + 1) * CH],
            ).then_inc(x_sems[u], 16)
            nc.gpsimd.dma_start(
                out=xs[:, h:, u * CH:(u + 1) * CH],
                in_=x_d[:, h:, u * CH:(u + 1) * CH],
            ).then_inc(x_sems[u], 16)
        # w DMAs per phase; ph>0 slot-t DMAs wait PE done with prior use.
        # PE waits the full-phase count (sound: later phases' DMAs are
        # blocked in-queue behind pe_sem gates).
        wcnt_slot = np.zeros((PH, TPP), np.int64)  # cumulative DMAs per slot
        run = [0] * TPP
        for ph in range(PH):
            for t in range(TPP):
                for a in range(2):
                    mm = int(m[a, ph, t])
                    if mm == 0:
                        continue
                    if ph > 0:
                        # slot t last used by round (ph-1, u=3, t)
                        last = (ph - 1) * (NU * TPP) + 3 * TPP + t
                        nc.sync.wait_ge(pe_sem, last + 1)
                    nc.sync.dma_start(
                        out=ws[64 * a:64 * (a + 1),
                               int(sb_off[a, t]):int(sb_off[a, t]) + mm, :],
                        in_=w_d[64 * a:64 * (a + 1),
                                int(d_off[a, ph, t]):int(d_off[a, ph, t]) + mm, :],
                    ).then_inc(w_sems[t], 16)
                    run[t] += 1
                wcnt_slot[ph, t] = run[t]

        # ---- PE warmup (junk, full-array) ----
        nc.vector.memset(wj[:], 0.0).then_inc(warm_sem, 1)
        nc.tensor.wait_ge(warm_sem, 1)
        for _ in range(400):
            nc.tensor.matmul(p0[:, :64], wj[:], wj[:, :64], start=True, stop=True,
                             skip_group_check=True)

        # ---- main loop ----
        rnd = 0
        for ph in range(PH):
            for u in range(NU):
                for t in range(TPP):
                    tg = ph * TPP + t
                    par = rnd % 4
                    # waits for this round (first MM of the round)
                    if u == 0:
                        nc.tensor.wait_ge(w_sems[t], 16 * int(wcnt_slot[ph, t]))
                    if ph == 0 and t == 0:
                        nc.tensor.wait_ge(x_sems[u], 32)
                    if rnd >= 4:
                        nc.tensor.wait_ge(dve_sem, 2 * (rnd - 3))
                    # lane cell lists + per-lane sbuf offsets
                    lanes = []
                    for a in range(2):
                        off = int(sb_off[a, t])
                        for c in range(NCG):
                            lst = cells[(tg, a, c)]
                            lanes.append((a, c, off, lst))
                            off += len(lst)
                    maxn = max(len(l[3]) for l in lanes)
                    inst = None
                    for i in range(maxn):
                        for a, c, off, lst in lanes:
                            if i >= len(lst):
                                continue
                            pidx = lst[i]
                            sl = int(slot_of[pidx])
                            inst = nc.tensor.matmul(
                                psb[par * 2 + a][32 * c:32 * (c + 1), :],
                                ws[64 * a:64 * (a + 1), off + i, :],
                                xs[64 * a:64 * (a + 1), sl, u * CH:(u + 1) * CH],
                                start=(i == 0), stop=(i == len(lst) - 1),
                                tile_position=(64 * a, 32 * c),
                                skip_group_check=True)
                    inst.then_inc(pe_sem, 1)

                    # eviction (DVE 2 instrs -> ACT bias -> gpsimd store)
                    par2 = rnd % 2
                    nc.vector.wait_ge(pe_sem, rnd + 1)
                    if rnd >= 2:
                        nc.vector.wait_ge(act_sem, rnd - 1)  # tb reuse
                    nc.vector.tensor_copy(
                        out=tb[:, par2, :], in_=psb[par * 2][:],
                    ).then_inc(dve_sem, 1)
                    nc.vector.wait_ge(dve_sem, 2 * rnd + 1)
                    nc.vector.tensor_add(
                        out=tb[:, par2, :], in0=psb[par * 2 + 1][:],
                        in1=tb[:, par2, :],
                    ).then_inc(dve_sem, 1)

                    nc.scalar.wait_ge(dve_sem, 2 * rnd + 2)
                    if rnd == 0:
                        nc.scalar.wait_ge(b_sem, 16)
                    if rnd >= 2:
                        # ob[par2] reuse: same-parity store of rnd-2 done
                        nc.scalar.wait_ge(st_sems[par2], 16 * (rnd // 2))
                    nc.scalar.activation(
                        out=ob[:, par2, :], in_=tb[:, par2, :],
                        func=mybir.ActivationFunctionType.Identity,
                        bias=bs[:, tg:tg + 1], scale=1.0,
                    ).then_inc(act_sem, 1)

                    nc.gpsimd.wait_ge(act_sem, rnd + 1)
                    nc.gpsimd.dma_start(
                        out=o_d[rnd], in_=ob[:, par2, :],
                    ).then_inc(st_sems[par2], 16)
                    rnd += 1

        nc.sync.wait_ge(st_sems[0], 16 * (NRND // 2))
        nc.sync.wait_ge(st_sems[1], 16 * (NRND // 2))
        nc.sync.drain()
        nc.all_engine_barrier()
    nc.compile()
    return nc, dict(m=m, d_off=d_off, WDTOT=WDTOT)


def _pack_w(plan, layout, weight, block_mask):
    pairs, rounds = plan["pairs"], plan["rounds"]
    cells, dummy = plan["cells"], plan["dummy"]
    m, d_off, WDTOT = layout["m"], layout["d_off"], layout["WDTOT"]

    mask = np.repeat(np.repeat(np.asarray(block_mask), BLOCK, 0), BLOCK, 1)
    w_eff = (2.0 * np.asarray(weight, np.float32)) * mask
    wT = np.ascontiguousarray(w_eff.T)        # [IN(k), OUT(o)]

    w_dev = np.zeros((P, WDTOT, BLOCK), np.float16)
    r32 = np.arange(BLOCK)
    for a in range(2):
        for ph in range(PH):
            for t in range(TPP):
                tg = ph * TPP + t
                off = int(d_off[a, ph, t])
                for c in range(NCG):
                    j = int(rounds[tg, c])
                    lst = cells[(tg, a, c)]
                    dum = dummy[(tg, a, c)]
                    ncell = len(lst)
                    if ncell == 0:
                        continue
                    pk = pairs[lst]            # [ncell, 2]
                    # karr [ncell, 64] absolute k rows
                    karr = np.concatenate(
                        [pk[:, 0:1] * BLOCK + r32, pk[:, 1:2] * BLOCK + r32], 1)
                    oarr = j * BLOCK + r32     # [32]
                    blk = wT[karr[:, :, None], oarr[None, None, :]]  # [ncell,64,32]
                    if dum:
                        blk = np.zeros_like(blk)
                    w_dev[64 * a:64 * (a + 1), off:off + ncell, :] = (
                        blk.transpose(1, 0, 2).astype(np.float16))
                    off += ncell
    return w_dev


def _pack_bias(plan, bias):
    rounds = plan["rounds"]
    b = np.asarray(bias, np.float32)
    b_dev = np.zeros((P, TG), np.float32)
    q = np.arange(BLOCK)
    for tg in range(TG):
        for c in range(NCG):
            j = int(rounds[tg, c])
            b_dev[32 * c + q, tg] = b[j * BLOCK + q]
    return b_dev


def _pack_x(plan, xb):
    # xb: [S, IN] f32 for one core -> [128, HALF, S] f16
    pairs, half, slot_of = plan["pairs"], plan["half"], plan["slot_of"]
    xT = xb.T                                  # [IN, S]
    kidx = np.zeros((P, HALF), np.int64)
    for p in range(NPAIR):
        a = int(half[p])
        s = int(slot_of[p])
        kidx[64 * a + np.arange(32), s] = pairs[p, 0] * BLOCK + np.arange(32)
        kidx[64 * a + 32 + np.arange(32), s] = pairs[p, 1] * BLOCK + np.arange(32)
    return np.ascontiguousarray(xT[kidx]).astype(np.float16)


def _unpack_out(plan, o_np):
    # o_np [NRND, 128, 512] f32 -> [S, OUT] f32
    rounds = plan["rounds"]
    y = np.empty((S, OUT), np.float32)
    rnd = 0
    for ph in range(PH):
        for u in range(NU):
            for t in range(TPP):
                tg = ph * TPP + t
                blockrows = o_np[rnd]          # [128 o, 512 s]
                for c in range(NCG):
                    j = int(rounds[tg, c])
                    y[u * CH:(u + 1) * CH, j * BLOCK:(j + 1) * BLOCK] = (
                        blockrows[32 * c:32 * (c + 1), :].T)
                rnd += 1
    return y


def _install_axon_ntff_hook(so_path="/opt/axon/libaxon_pjrt.so"):
    """NTFF profiling via ctypes when antenv lacks axon_hooks."""
    import contextlib
    import ctypes
    import sys
    import types

    lib = ctypes.CDLL(so_path)
    if not hasattr(lib, "axon_start_nrt_profile"):
        return
    lib.axon_start_nrt_profile.argtypes = [
        ctypes.POINTER(ctypes.c_int64),
        ctypes.c_size_t,
    ]
    lib.axon_start_nrt_profile.restype = ctypes.c_int64
    lib.axon_stop_nrt_profile.argtypes = [ctypes.c_char_p]
    lib.axon_stop_nrt_profile.restype = ctypes.c_int64

    @contextlib.contextmanager
    def _hook(output_dir, device_ids):
        import jax

        jax.devices()
        if device_ids:
            ids = (ctypes.c_int64 * len(device_ids))(*device_ids)
            rc = lib.axon_start_nrt_profile(ids, len(device_ids))
        else:
            rc = lib.axon_start_nrt_profile(None, 0)
        if rc != 0:
            raise RuntimeError(f"axon_start_nrt_profile rc={rc}")
        try:
            yield
        finally:
            n = lib.axon_stop_nrt_profile(str(output_dir).encode())
            print(f"ntff profile: {n} file(s) -> {output_dir}", file=sys.stderr)

    mod = types.ModuleType("antenv.axon_hooks")
    mod.get_axon_ntff_profile_hook = lambda: _hook
    mod.set_axon_ntff_profile_hook = lambda h: None
    sys.modules["antenv.axon_hooks"] = mod

    import concourse.bass_utils as bu

    bu.upload_artifacts = lambda tmpdir: f"file://{tmpdir}"


def kernel(x, weight, bias, block_mask):
    global LAST_EXEC_NS
    import os
    from concourse.bass_utils import run_bass_kernel_spmd

    plan = _plan(np.asarray(block_mask))
    nc, layout = _build_program(plan)

    w_dev = _pack_w(plan, layout, weight, block_mask)
    b_dev = _pack_bias(plan, bias)
    xs = np.asarray(x, np.float32)
    in_maps = [
        {"xt": _pack_x(plan, xs[b]), "wt": w_dev, "bias": b_dev}
        for b in range(B)
    ]

    trace = bool(int(os.environ.get("BSL_TRACE", "0")))
    if trace:
        _install_axon_ntff_hook()
    res = run_bass_kernel_spmd(nc, in_maps, list(range(B)), trace=trace)
    LAST_EXEC_NS = res.exec_time_ns
    return np.stack(
        [_unpack_out(plan, res.results[b]["out"]) for b in range(B)]
    ).astype(np.float32)


# revision 5
# speedup vs baseline: 1.0877x; 1.0877x over previous
"""Block-sparse linear kernel for Trainium2 — 8-core data-parallel,
PE-array tiling (8 tiles of 64x32) exploiting the 32x32 block mask.

out = 2*(x @ (weight*mask).T) + bias
x: (8, 2048, 4096) f32, weight: (4096, 4096) f32, bias: (4096,),
block_mask: (128, 128) bool over 32x32 blocks (~50% dense).

Each core handles one batch (M=2048, K=4096, N=4096). k-blocks are paired
(greedy max matching on shared-zero o-columns) into 64 K=64 "pair" units
split across the two PE row-halves; o-blocks are grouped 4-per-round into
32 j-rounds (balanced by local search). Only (pair, o-block) cells with at
least one unmasked member are computed: ~69% of the dense work. The PE runs
in 64x32 tile mode (8 concurrent tiles = row-half x col-group), raw bass
emission with ~1 semaphore per round instead of per-instruction.
PSUM: 8 banks as 4-deep round rotation x 2 row-halves. Eviction: DVE sums
the two half-banks, ACT adds bias (Identity activation, per-partition bias)
and the result is stored [o, s]-transposed; the host reassembles.
"""
import numpy as np

B, S, IN, OUT = 8, 2048, 4096, 4096
BLOCK = 32
P = 128
NJ = OUT // BLOCK          # 128 o-blocks
NK = IN // BLOCK           # 128 k-blocks
NPAIR = NK // 2            # 64 k-pairs
HALF = NPAIR // 2          # 32 pairs per PE row-half
NCG = 4                    # col-groups (o-blocks per round)
TG = NJ // NCG             # 32 j-rounds
PH = 4                     # phases
TPP = TG // PH             # 8 j-rounds per phase
NU = 4                     # s-chunks
CH = S // NU               # 512
NRND = PH * NU * TPP       # 128 (ph, u, t) rounds

LAST_EXEC_NS = None


def _plan(bm):
    """bm: [128 o-blocks, 128 k-blocks] bool -> schedule plan."""
    bm = bm.astype(np.int8)
    rng = np.random.default_rng(0)

    # --- pair k-blocks: greedy max matching on #shared-zero o-columns ---
    z = (1 - bm.T).astype(np.int32)          # [k, o]
    agree = z @ z.T
    np.fill_diagonal(agree, -1)
    order = np.dstack(np.unravel_index(np.argsort(-agree, axis=None), agree.shape))[0]
    used = np.zeros(NK, bool)
    pairs = []
    for k1, k2 in order:
        if used[k1] or used[k2]:
            continue
        used[k1] = used[k2] = True
        pairs.append((int(k1), int(k2)))
    pairs = np.array(pairs)                   # [64, 2]

    # refine matching: pair-member swaps reducing total cells
    def _ncells(P):
        return int(((bm[:, P[:, 0]] | bm[:, P[:, 1]]) > 0).sum())

    cur = _ncells(pairs)
    rng2 = np.random.default_rng(1)
    for _ in range(60000):
        p1, p2 = rng2.integers(0, NPAIR, 2)
        if p1 == p2:
            continue
        m1, m2 = rng2.integers(0, 2, 2)
        Pn = pairs.copy()
        Pn[p1, m1], Pn[p2, m2] = pairs[p2, m2], pairs[p1, m1]
        c = _ncells(Pn)
        if c <= cur:
            pairs, cur = Pn, c

    # cell[j, p] = 1 if pair p needed for o-block j
    cell = ((bm[:, pairs[:, 0]] | bm[:, pairs[:, 1]]) > 0).astype(np.int64)  # [j, pair]

    # --- assign pairs to halves + group j's into rounds (local search) ---
    half = np.zeros(NPAIR, np.int32)
    half[np.argsort(-cell.sum(0))] = np.arange(NPAIR) % 2
    nj = cell.sum(1)
    rounds = np.argsort(-nj).reshape(TG, NCG).copy()

    def lane_n(h):
        # n[j, a] = #pairs in half a needed by j
        return np.stack([cell[:, h == a].sum(1) for a in range(2)], 1)

    n = lane_n(half)
    cost = n[rounds].max(axis=(1, 2)).sum()
    for it in range(40000):
        if it % 2 == 0:
            p1, p2 = rng.integers(0, NPAIR, 2)
            a1, a2 = half[p1], half[p2]
            if a1 == a2:
                continue
            dn = np.zeros((NJ, 2), np.int64)
            dn[:, a1] = cell[:, p2] - cell[:, p1]
            dn[:, a2] = cell[:, p1] - cell[:, p2]
            n2 = n + dn
            c2 = n2[rounds].max(axis=(1, 2)).sum()
            if c2 <= cost:
                half[p1], half[p2] = a2, a1
                n, cost = n2, c2
        else:
            t1, t2 = rng.integers(0, TG, 2)
            if t1 == t2:
                continue
            c1i, c2i = rng.integers(0, NCG, 2)
            r2 = rounds.copy()
            r2[t1, c1i], r2[t2, c2i] = rounds[t2, c2i], rounds[t1, c1i]
            c2 = n[r2].max(axis=(1, 2)).sum()
            if c2 <= cost:
                rounds, cost = r2, c2

    # slots: pair -> (half a, slot index within half)
    slot_of = np.zeros(NPAIR, np.int32)
    for a in range(2):
        idx = np.where(half == a)[0]
        slot_of[idx] = np.arange(len(idx))

    # per (tg, a, c): list of pair indices (cells); pad empty lanes with a
    # dummy (first pair of the half, weights zeroed at pack time)
    cells = {}
    dummy = {}
    for tg in range(TG):
        for a in range(2):
            hp = np.where(half == a)[0]
            for c in range(NCG):
                j = rounds[tg, c]
                lst = [int(p) for p in hp if cell[j, p]]
                dum = False
                if not lst:
                    lst = [int(hp[0])]
                    dum = True
                cells[(tg, a, c)] = lst
                dummy[(tg, a, c)] = dum
    return dict(pairs=pairs, half=half, rounds=rounds, slot_of=slot_of,
                cells=cells, dummy=dummy)


def _build_program(plan):
    import concourse.bacc as bacc
    from concourse import mybir

    f16 = mybir.dt.float16
    f32 = mybir.dt.float32
    pairs, rounds = plan["pairs"], plan["rounds"]
    slot_of, cells = plan["slot_of"], plan["cells"]

    # per (a): sbuf w slot capacities per round-slot t (reused across phases)
    m = np.zeros((2, PH, TPP), np.int64)     # cells per (a, ph, t)
    for ph in range(PH):
        for t in range(TPP):
            tg = ph * TPP + t
            for a in range(2):
                m[a, ph, t] = sum(len(cells[(tg, a, c)]) for c in range(NCG))
    cap = m.max(axis=1)                       # [2, TPP]
    sb_off = np.zeros((2, TPP), np.int64)     # sbuf cell offset per (a, t)
    for a in range(2):
        sb_off[a] = np.concatenate([[0], np.cumsum(cap[a])[:-1]])
    WCAP = int(cap.sum(axis=1).max())
    # dram w offsets per (a, ph, t)
    d_off = np.zeros((2, PH, TPP), np.int64)
    for a in range(2):
        flat = m[a].reshape(-1)
        d_off[a] = np.concatenate([[0], np.cumsum(flat)[:-1]]).reshape(PH, TPP)
    WDTOT = int(m.sum(axis=(1, 2)).max())

    nc = bacc.Bacc("TRN2", debug=False, num_devices=B)
    x_d = nc.dram_tensor("xt", (P, HALF, S), f16, kind="ExternalInput")
    w_d = nc.dram_tensor("wt", (P, WDTOT, BLOCK), f16, kind="ExternalInput")
    b_d = nc.dram_tensor("bias", (P, TG), f32, kind="ExternalInput")
    o_d = nc.dram_tensor("out", (NRND, P, CH), f32, kind="ExternalOutput")

    x_sems = [nc.alloc_semaphore(f"xsem{u}") for u in range(NU)]
    xg_sems = [nc.alloc_semaphore(f"xgsem{u}") for u in range(NU)]
    w_sems = [nc.alloc_semaphore(f"wsem{t}") for t in range(TPP)]
    b_sem = nc.alloc_semaphore("bsem")
    warm_sem = nc.alloc_semaphore("warmsem")
    pe_sem = nc.alloc_semaphore("pesem")
    dve_sem = nc.alloc_semaphore("dvesem")
    act_sem = nc.alloc_semaphore("actsem")
    st_sems = [nc.alloc_semaphore(f"stsem{p}") for p in range(2)]

    with (
        nc.sbuf_tensor([P, HALF, S], f16) as xs,
        nc.sbuf_tensor([P, WCAP, BLOCK], f16) as ws,
        nc.sbuf_tensor([P, TG], f32) as bs,
        nc.sbuf_tensor([P, P], f16) as wj,
        nc.sbuf_tensor([P, 2, CH], f32) as tb,
        nc.sbuf_tensor([P, 2, CH], f32) as ob,
        nc.psum_tensor([P, CH], f32) as p0,
        nc.psum_tensor([P, CH], f32) as p1,
        nc.psum_tensor([P, CH], f32) as p2,
        nc.psum_tensor([P, CH], f32) as p3,
        nc.psum_tensor([P, CH], f32) as p4,
        nc.psum_tensor([P, CH], f32) as p5,
        nc.psum_tensor([P, CH], f32) as p6,
        nc.psum_tensor([P, CH], f32) as p7,
    ):
        psb = [p0, p1, p2, p3, p4, p5, p6, p7]

        # ---- prologue DMAs ----
        nc.sync.dma_start(out=bs[:], in_=b_d[:]).then_inc(b_sem, 16)
        # x chunks on the scalar queue (parallel with w on sync)
        for u in range(NU):
            h = HALF // 2
            nc.scalar.dma_start(
                out=xs[:, :h, u * CH:(u + 1) * CH],
                in_=x_d[:, :h, u * CH:(u + 1) * CH],
            ).then_inc(x_sems[u], 16)
            nc.gpsimd.dma_start(
                out=xs[:, h:, u * CH:(u + 1) * CH],
                in_=x_d[:, h:, u * CH:(u + 1) * CH],
            ).then_inc(xg_sems[u], 16)
        # w DMAs per phase; ph>0 slot-t DMAs wait PE done with prior use.
        # PE waits the full-phase count (sound: later phases' DMAs are
        # blocked in-queue behind pe_sem gates).
        wcnt_slot = np.zeros((PH, TPP), np.int64)  # cumulative DMAs per slot
        run = [0] * TPP
        for ph in range(PH):
            for t in range(TPP):
                for a in range(2):
                    mm = int(m[a, ph, t])
                    if mm == 0:
                        continue
                    if ph > 0:
                        # slot t last used by round (ph-1, u=3, t)
                        last = (ph - 1) * (NU * TPP) + 3 * TPP + t
                        nc.sync.wait_ge(pe_sem, last + 1)
                    nc.sync.dma_start(
                        out=ws[64 * a:64 * (a + 1),
                               int(sb_off[a, t]):int(sb_off[a, t]) + mm, :],
                        in_=w_d[64 * a:64 * (a + 1),
                                int(d_off[a, ph, t]):int(d_off[a, ph, t]) + mm, :],
                    ).then_inc(w_sems[t], 16)
                    run[t] += 1
                wcnt_slot[ph, t] = run[t]

        # ---- PE warmup (junk, full-array) ----
        nc.vector.memset(wj[:], 0.0).then_inc(warm_sem, 1)
        nc.tensor.wait_ge(warm_sem, 1)
        for _ in range(400):
            nc.tensor.matmul(p0[:, :64], wj[:], wj[:, :64], start=True, stop=True,
                             skip_group_check=True)

        # ---- main loop ----
        rnd = 0
        for ph in range(PH):
            for u in range(NU):
                for t in range(TPP):
                    tg = ph * TPP + t
                    par = rnd % 4
                    # waits for this round (first MM of the round)
                    if u == 0:
                        nc.tensor.wait_ge(w_sems[t], 16 * int(wcnt_slot[ph, t]))
                    if ph == 0 and t == 0:
                        nc.tensor.wait_ge(x_sems[u], 16)
                        nc.tensor.wait_ge(xg_sems[u], 16)
                    if rnd >= 4:
                        nc.tensor.wait_ge(dve_sem, 2 * (rnd - 3))
                    # lane cell lists + per-lane sbuf offsets
                    lanes = []
                    for a in range(2):
                        off = int(sb_off[a, t])
                        for c in range(NCG):
                            lst = cells[(tg, a, c)]
                            lanes.append((a, c, off, lst))
                            off += len(lst)
                    maxn = max(len(l[3]) for l in lanes)
                    inst = None
                    for i in range(maxn):
                        for a, c, off, lst in lanes:
                            if i >= len(lst):
                                continue
                            pidx = lst[i]
                            sl = int(slot_of[pidx])
                            inst = nc.tensor.matmul(
                                psb[par * 2 + a][32 * c:32 * (c + 1), :],
                                ws[64 * a:64 * (a + 1), off + i, :],
                                xs[64 * a:64 * (a + 1), sl, u * CH:(u + 1) * CH],
                                start=(i == 0), stop=(i == len(lst) - 1),
                                tile_position=(64 * a, 32 * c),
                                skip_group_check=True)
                    inst.then_inc(pe_sem, 1)

                    # eviction (DVE 2 instrs -> ACT bias -> gpsimd store)
                    par2 = rnd % 2
                    nc.vector.wait_ge(pe_sem, rnd + 1)
                    if rnd >= 2:
                        nc.vector.wait_ge(act_sem, rnd - 1)  # tb reuse
                    nc.vector.tensor_copy(
                        out=tb[:, par2, :], in_=psb[par * 2][:],
                    ).then_inc(dve_sem, 1)
                    nc.vector.wait_ge(dve_sem, 2 * rnd + 1)
                    nc.vector.tensor_add(
                        out=tb[:, par2, :], in0=psb[par * 2 + 1][:],
                        in1=tb[:, par2, :],
                    ).then_inc(dve_sem, 1)

                    nc.scalar.wait_ge(dve_sem, 2 * rnd + 2)
                    if rnd == 0:
                        nc.scalar.wait_ge(b_sem, 16)
                    if rnd >= 2:
                        # ob[par2] reuse: same-parity store of rnd-2 done
                        nc.scalar.wait_ge(st_sems[par2], 16 * (rnd // 2))
                    nc.scalar.activation(
                        out=ob[:, par2, :], in_=tb[:, par2, :],
                        func=mybir.ActivationFunctionType.Identity,
                        bias=bs[:, tg:tg + 1], scale=1.0,
                    ).then_inc(act_sem, 1)

                    nc.gpsimd.wait_ge(act_sem, rnd + 1)
                    nc.gpsimd.dma_start(
                        out=o_d[rnd], in_=ob[:, par2, :],
                    ).then_inc(st_sems[par2], 16)
                    rnd += 1

        nc.sync.wait_ge(st_sems[0], 16 * (NRND // 2))
        nc.sync.wait_ge(st_sems[1], 16 * (NRND // 2))
        nc.sync.drain()
        nc.all_engine_barrier()
    nc.compile()
    return nc, dict(m=m, d_off=d_off, WDTOT=WDTOT)


def _pack_w(plan, layout, weight, block_mask):
    pairs, rounds = plan["pairs"], plan["rounds"]
    cells, dummy = plan["cells"], plan["dummy"]
    m, d_off, WDTOT = layout["m"], layout["d_off"], layout["WDTOT"]

    mask = np.repeat(np.repeat(np.asarray(block_mask), BLOCK, 0), BLOCK, 1)
    w_eff = (2.0 * np.asarray(weight, np.float32)) * mask
    wT = np.ascontiguousarray(w_eff.T)        # [IN(k), OUT(o)]

    w_dev = np.zeros((P, WDTOT, BLOCK), np.float16)
    r32 = np.arange(BLOCK)
    for a in range(2):
        for ph in range(PH):
            for t in range(TPP):
                tg = ph * TPP + t
                off = int(d_off[a, ph, t])
                for c in range(NCG):
                    j = int(rounds[tg, c])
                    lst = cells[(tg, a, c)]
                    dum = dummy[(tg, a, c)]
                    ncell = len(lst)
                    if ncell == 0:
                        continue
                    pk = pairs[lst]            # [ncell, 2]
                    # karr [ncell, 64] absolute k rows
                    karr = np.concatenate(
                        [pk[:, 0:1] * BLOCK + r32, pk[:, 1:2] * BLOCK + r32], 1)
                    oarr = j * BLOCK + r32     # [32]
                    blk = wT[karr[:, :, None], oarr[None, None, :]]  # [ncell,64,32]
                    if dum:
                        blk = np.zeros_like(blk)
                    w_dev[64 * a:64 * (a + 1), off:off + ncell, :] = (
                        blk.transpose(1, 0, 2).astype(np.float16))
                    off += ncell
    return w_dev


def _pack_bias(plan, bias):
    rounds = plan["rounds"]
    b = np.asarray(bias, np.float32)
    b_dev = np.zeros((P, TG), np.float32)
    q = np.arange(BLOCK)
    for tg in range(TG):
        for c in range(NCG):
            j = int(rounds[tg, c])
            b_dev[32 * c + q, tg] = b[j * BLOCK + q]
    return b_dev


def _pack_x(plan, xb):
    # xb: [S, IN] f32 for one core -> [128, HALF, S] f16
    pairs, half, slot_of = plan["pairs"], plan["half"], plan["slot_of"]
    xT = xb.T                                  # [IN, S]
    kidx = np.zeros((P, HALF), np.int64)
    for p in range(NPAIR):
        a = int(half[p])
        s = int(slot_of[p])
        kidx[64 * a + np.arange(32), s] = pairs[p, 0] * BLOCK + np.arange(32)
        kidx[64 * a + 32 + np.arange(32), s] = pairs[p, 1] * BLOCK + np.arange(32)
    return np.ascontiguousarray(xT[kidx]).astype(np.float16)


def _unpack_out(plan, o_np):
    # o_np [NRND, 128, 512] f32 -> [S, OUT] f32
    rounds = plan["rounds"]
    y = np.empty((S, OUT), np.float32)
    rnd = 0
    for ph in range(PH):
        for u in range(NU):
            for t in range(TPP):
                tg = ph * TPP + t
                blockrows = o_np[rnd]          # [128 o, 512 s]
                for c in range(NCG):
                    j = int(rounds[tg, c])
                    y[u * CH:(u + 1) * CH, j * BLOCK:(j + 1) * BLOCK] = (
                        blockrows[32 * c:32 * (c + 1), :].T)
                rnd += 1
    return y


def _install_axon_ntff_hook(so_path="/opt/axon/libaxon_pjrt.so"):
    """NTFF profiling via ctypes when antenv lacks axon_hooks."""
    import contextlib
    import ctypes
    import sys
    import types

    lib = ctypes.CDLL(so_path)
    if not hasattr(lib, "axon_start_nrt_profile"):
        return
    lib.axon_start_nrt_profile.argtypes = [
        ctypes.POINTER(ctypes.c_int64),
        ctypes.c_size_t,
    ]
    lib.axon_start_nrt_profile.restype = ctypes.c_int64
    lib.axon_stop_nrt_profile.argtypes = [ctypes.c_char_p]
    lib.axon_stop_nrt_profile.restype = ctypes.c_int64

    @contextlib.contextmanager
    def _hook(output_dir, device_ids):
        import jax

        jax.devices()
        if device_ids:
            ids = (ctypes.c_int64 * len(device_ids))(*device_ids)
            rc = lib.axon_start_nrt_profile(ids, len(device_ids))
        else:
            rc = lib.axon_start_nrt_profile(None, 0)
        if rc != 0:
            raise RuntimeError(f"axon_start_nrt_profile rc={rc}")
        try:
            yield
        finally:
            n = lib.axon_stop_nrt_profile(str(output_dir).encode())
            print(f"ntff profile: {n} file(s) -> {output_dir}", file=sys.stderr)

    mod = types.ModuleType("antenv.axon_hooks")
    mod.get_axon_ntff_profile_hook = lambda: _hook
    mod.set_axon_ntff_profile_hook = lambda h: None
    sys.modules["antenv.axon_hooks"] = mod

    import concourse.bass_utils as bu

    bu.upload_artifacts = lambda tmpdir: f"file://{tmpdir}"


def kernel(x, weight, bias, block_mask):
    global LAST_EXEC_NS
    import os
    from concourse.bass_utils import run_bass_kernel_spmd

    plan = _plan(np.asarray(block_mask))
    nc, layout = _build_program(plan)

    w_dev = _pack_w(plan, layout, weight, block_mask)
    b_dev = _pack_bias(plan, bias)
    xs = np.asarray(x, np.float32)
    in_maps = [
        {"xt": _pack_x(plan, xs[b]), "wt": w_dev, "bias": b_dev}
        for b in range(B)
    ]

    trace = bool(int(os.environ.get("BSL_TRACE", "0")))
    if trace:
        _install_axon_ntff_hook()
    res = run_bass_kernel_spmd(nc, in_maps, list(range(B)), trace=trace)
    LAST_EXEC_NS = res.exec_time_ns
    return np.stack(
        [_unpack_out(plan, res.results[b]["out"]) for b in range(B)]
    ).astype(np.float32)
